# revision 1
# baseline (speedup 1.0000x reference)
"""CosineTransformerBlock Trainium2 kernel (8 NeuronCores, SPMD, no collectives).

Sharding: core c handles batch b = c // 2 and query-token rows
[ (c % 2) * 1024 : (c % 2) * 1024 + 1024 ] of that batch.  K/V work for a
batch is duplicated across the 2 cores that share it (cheaper than pair
collectives on this chip).

Key algebraic transform: cosine attention has no softmax, so
    (qn @ kn^T) @ v  ==  qn @ (kn^T @ v)
which turns the O(N^2) attention into two tiny per-head [64,64] matmuls.

Layout strategy (per core):
  - activations are token-major [tok, feat] so LayerNorm / l2-norm use
    bn_stats + per-partition scalar ops;
  - matmul lhsT operands are produced by casting to bf16 and XBAR
    DMA-transposing 128x128 blocks (free: runs on DMA engines);
  - all matmuls are bf16 with fp32 PSUM accumulation;
  - LN affine (g, b) is folded into the following weight matrix on the host:
      LN(x) @ W = std(x) @ (g[:, None] * W) + (b @ W)
    the (b @ W) row term is added via a K=1 ones-matmul into the same PSUM
    accumulation group (emitted only when the row is nonzero).
"""

import os
import sys

sys.path.insert(0, "/opt/trn_rl_repo")

import numpy as np
import ml_dtypes

# ---- problem shapes (hardcoded per contract) ----
B, N, D = 4, 2048, 1024
H, DH = 16, 64
INNER = H * DH  # 1024
MLP = 4096
EPS = 1e-5
NCORES = 8
TQ = N // 2  # 1024 query tokens per core
TKV = N  # 2048 kv tokens per core
P = 128
DC = D // P  # 8 chunks of the model dim
IC = INNER // P  # 8
MC = MLP // P  # 32
NQT = TQ // P  # 8 q token tiles
NKT = TKV // P  # 16 kv token tiles

BF16 = None  # set lazily (mybir import)
F32 = None


def _dt():
    global BF16, F32
    import concourse.mybir as mybir

    BF16 = mybir.dt.bfloat16
    F32 = mybir.dt.float32
    return mybir


def _ln_stats_ops(nc, pool, x_tile, ntok, dfree, eps_tile):
    """bn_stats/bn_aggr over free dim -> (rs, neg_mu_rs) [ntok,1] fp32."""
    import concourse.mybir as mybir

    nsub = (dfree + 511) // 512
    sub = dfree // nsub
    stats = pool.tile([P, nsub, 6], F32, tag="ln_stats")
    xv = x_tile.rearrange("p (s f) -> p s f", s=nsub)
    for s in range(nsub):
        nc.vector.bn_stats(out=stats[:ntok, s, :], in_=xv[:ntok, s, :])
    mv = pool.tile([P, 2], F32, tag="ln_mv")
    nc.vector.bn_aggr(out=mv[:ntok], in_=stats[:ntok])
    rs = pool.tile([P, 1], F32, tag="ln_rs")
    # rs = 1/sqrt(var + eps)
    nc.scalar.activation(
        out=rs[:ntok],
        in_=mv[:ntok, 1:2],
        func=mybir.ActivationFunctionType.Sqrt,
        bias=eps_tile[:ntok],
        scale=1.0,
    )
    nc.vector.reciprocal(out=rs[:ntok], in_=rs[:ntok])
    nmu = pool.tile([P, 1], F32, tag="ln_nmu")
    # nmu = -mu * rs
    nc.vector.tensor_scalar(
        out=nmu[:ntok],
        in0=mv[:ntok, 0:1],
        scalar1=rs[:ntok],
        scalar2=-1.0,
        op0=mybir.AluOpType.mult,
        op1=mybir.AluOpType.mult,
    )
    return rs, nmu


def build_nc(bias_rows):
    """Build the SPMD program. bias_rows: dict of host-computed fp32 rows
    (bq, bk, bv, bo, b2: [dim] arrays) - a K=1 ones-matmul is emitted for
    each nonzero row."""
    mybir = _dt()
    import concourse.bass as bass
    import concourse.tile as tile
    from concourse import bacc

    AF = mybir.ActivationFunctionType
    ALU = mybir.AluOpType

    nc = bacc.Bacc("TRN2", target_bir_lowering=False, debug=False, num_devices=NCORES)

    # ---- DRAM I/O ----
    Qd = nc.dram_tensor("q_tok", [TQ, D], F32, kind="ExternalInput").ap()
    Kd = nc.dram_tensor("k_tok", [TKV, D], BF16, kind="ExternalInput").ap()
    Vd = nc.dram_tensor("v_tok", [TKV, D], BF16, kind="ExternalInput").ap()
    wq_d = nc.dram_tensor("wq", [D, INNER], BF16, kind="ExternalInput").ap()
    wk_d = nc.dram_tensor("wk", [D, INNER], BF16, kind="ExternalInput").ap()
    wv_d = nc.dram_tensor("wv", [D, INNER], BF16, kind="ExternalInput").ap()
    wo_d = nc.dram_tensor("wo", [INNER, D], BF16, kind="ExternalInput").ap()
    w1_d = nc.dram_tensor("w1", [MC * P, DC * P], BF16, kind="ExternalInput").ap()
    w2_d = nc.dram_tensor("w2", [MLP, D], BF16, kind="ExternalInput").ap()
    bff1_d = nc.dram_tensor("bff1", [P, MC], F32, kind="ExternalInput").ap()
    brow_d = {}
    for name in ("bq", "bk", "bv", "bo", "b2"):
        if np.any(bias_rows[name]):
            brow_d[name] = nc.dram_tensor(
                "brow_" + name, [1, bias_rows[name].shape[0]], BF16,
                kind="ExternalInput",
            ).ap()
    Yd = nc.dram_tensor("y", [TQ, D], F32, kind="ExternalOutput").ap()
    DBG = bool(os.environ.get("BASS_DEBUG_KERNEL"))
    dbg = {}
    if DBG:
        for nm, shp in [("dbg_kn", [P, INNER]), ("dbg_v", [P, INNER]),
                        ("dbg_M", [P, IC * DH]), ("dbg_x", [P, D]),
                        ("dbg_qn", [P, INNER]), ("dbg_aT", [P, IC * P])]:
            dbg[nm] = nc.dram_tensor(nm, shp, F32, kind="ExternalOutput").ap()

    Qt = Qd.rearrange("(t p) d -> t p d", p=P)
    Kt = Kd.rearrange("(t p) d -> t p d", p=P)
    Vt = Vd.rearrange("(t p) d -> t p d", p=P)
    Yt = Yd.rearrange("(t p) d -> t p d", p=P)
    # weight DRAM views: [P, chunk, cols]
    wq_v = wq_d.rearrange("(c p) n -> p c n", p=P)
    wk_v = wk_d.rearrange("(c p) n -> p c n", p=P)
    wv_v = wv_d.rearrange("(c p) n -> p c n", p=P)
    wo_v = wo_d.rearrange("(c p) n -> p c n", p=P)
    w1_v = w1_d.rearrange("(m p) (c q) -> m p c q", p=P, c=DC)
    w2_v = w2_d.rearrange("(c p) n -> p c n", p=P)

    with tile.TileContext(nc) as tc:
        with tc.tile_pool(name="singles", bufs=1) as singles:
            # resident weights
            wq_sb = singles.tile([P, DC, INNER], BF16)
            wk_sb = singles.tile([P, DC, INNER], BF16)
            wv_sb = singles.tile([P, DC, INNER], BF16)
            wo_sb = singles.tile([P, IC, D], BF16)
            for c in range(DC):
                nc.gpsimd.dma_start(wk_sb[:, c, :], wk_v[:, c, :])
            nc.sync.dma_start(wv_sb[:], wv_v[:])
            nc.gpsimd.dma_start(wq_sb[:], wq_v[:])
            nc.gpsimd.dma_start(wo_sb[:], wo_v[:])
            bff1_sb = singles.tile([P, MC], F32)
            nc.sync.dma_start(bff1_sb[:], bff1_d[:])
            eps_tile = singles.tile([P, 1], F32)
            nc.vector.memset(eps_tile[:], EPS)
            ones_row = singles.tile([1, P], BF16)
            nc.vector.memset(ones_row[:], 1.0)
            brow_sb = {}
            for name, ap in brow_d.items():
                t = singles.tile([1, ap.shape[1]], BF16, tag="brow_" + name)
                nc.sync.dma_start(t[:], ap[:])
                brow_sb[name] = t
            # residual / LN2 source
            x_sb = singles.tile([P, NQT, D], F32)
            # head-pair attention matrices: M_sb[:, pr, :] is
            # blockdiag(M_2pr, M_2pr+1); off-diagonal junk stays zero
            M_sb = singles.tile([P, IC, P], BF16)
            nc.vector.memset(M_sb[:], 0.0)

            def bias_mm(ps, name, lo, hi, start):
                """Accumulate bias row[lo:hi] into psum ps via K=1 matmul."""
                if name in brow_sb:
                    nc.tensor.matmul(
                        ps,
                        ones_row[:, : ps.shape[0]],
                        brow_sb[name][:, lo:hi],
                        start=start,
                        stop=False,
                        skip_group_check=True,
                    )
                    return False  # start consumed
                return start

            # ---------------- Phase 1: K/V -> M_h ----------------
            with (
                tc.tile_pool(name="kv_io", bufs=4) as kv_io,
                tc.tile_pool(name="kv_mid", bufs=3) as kv_mid,
                tc.tile_pool(name="kv_stats", bufs=4) as kv_stats,
                tc.tile_pool(name="kv_ps", bufs=6, space="PSUM") as kv_ps,
                tc.tile_pool(name="m_ps", bufs=1, space="PSUM") as m_ps_pool,
            ):
                M_ps = m_ps_pool.tile([P, IC, P], F32)
                for t in range(NKT):
                    kn_bf = None
                    v_bf = None
                    for which in ("k", "v"):
                        src = Kt[t] if which == "k" else Vt[t]
                        w_sb = wk_sb if which == "k" else wv_sb
                        bname = "bk" if which == "k" else "bv"
                        x_in = kv_io.tile([P, D], BF16, tag="kv_in")
                        nc.sync.dma_start(x_in[:], src[:])
                        rs, nmu = _ln_stats_ops(nc, kv_stats, x_in, P, D, eps_tile)
                        xn = kv_mid.tile([P, D], BF16, tag="kv_std")
                        nc.scalar.activation(
                            out=xn[:], in_=x_in[:], func=AF.Identity, bias=nmu[:], scale=rs[:]
                        )
                        xnT = kv_mid.tile([P, DC, P], BF16, tag="kv_xnT")
                        for c in range(DC):
                            nc.sync.dma_start(
                                xnT[:, c, :], xn[:, c * P : (c + 1) * P], transpose=True
                            )
                        # projection: [tok, INNER] in 2 groups of 512
                        pss = []
                        for g in range(2):
                            ps = kv_ps.tile([P, 512], F32, tag="kv_proj")
                            pss.append(ps)
                        for c in range(DC):
                            for g in range(2):
                                nc.tensor.matmul(
                                    pss[g][:],
                                    xnT[:, c, :],
                                    w_sb[:, c, g * 512 : (g + 1) * 512],
                                    start=(c == 0),
                                    stop=(c == DC - 1) and (bname not in brow_sb),
                                )
                        for g in range(2):
                            if bname in brow_sb:
                                nc.tensor.matmul(
                                    pss[g][:],
                                    ones_row[:],
                                    brow_sb[bname][:, g * 512 : (g + 1) * 512],
                                    start=False,
                                    stop=True,
                                    skip_group_check=True,
                                )
                        if which == "v":
                            v_bf = kv_mid.tile([P, INNER], BF16, tag="v_bf")
                            for g in range(2):
                                nc.scalar.activation(
                                    out=v_bf[:, g * 512 : (g + 1) * 512],
                                    in_=pss[g][:],
                                    func=AF.Copy,
                                )
                        else:
                            # l2-normalize per head
                            kn_bf = kv_mid.tile([P, H, DH], BF16, tag="kn_bf")
                            for g in range(2):
                                sq = kv_mid.tile([P, 512], F32, tag="kv_sq")
                                nc.scalar.activation(
                                    out=sq[:], in_=pss[g][:], func=AF.Square
                                )
                                ss = kv_stats.tile([P, 8, 1], F32, tag="l2_ss")
                                nc.vector.reduce_sum(
                                    out=ss[:],
                                    in_=sq.rearrange("p (h f) -> p h f", h=8),
                                    axis=mybir.AxisListType.X,
                                )
                                rn = kv_stats.tile([P, 8, 1], F32, tag="l2_rn")
                                nc.scalar.activation(
                                    out=rn[:], in_=ss[:], func=AF.Sqrt
                                )
                                nc.vector.tensor_scalar_max(
                                    out=rn[:], in0=rn[:], scalar1=1e-12
                                )
                                nc.vector.reciprocal(out=rn[:], in_=rn[:])
                                nc.vector.tensor_tensor(
                                    out=kn_bf[:, g * 8 : (g + 1) * 8, :],
                                    in0=pss[g].rearrange("p (h f) -> p h f", h=8),
                                    in1=rn.to_broadcast([P, 8, DH]),
                                    op=ALU.mult,
                                )
                    if DBG and t == 0:
                        knf = kv_mid.tile([P, INNER], F32, tag="dbg_knf")
                        nc.vector.tensor_copy(out=knf.rearrange("p (h f) -> p h f", h=H), in_=kn_bf[:])
                        nc.sync.dma_start(dbg["dbg_kn"][:], knf[:])
                        vf = kv_mid.tile([P, INNER], F32, tag="dbg_vf")
                        nc.vector.tensor_copy(out=vf[:], in_=v_bf[:])
                        nc.sync.dma_start(dbg["dbg_v"][:], vf[:])
                    # M_h accumulation: M[h] += kn_h^T @ v_h
                    # Heads are processed in pairs: one [128,128] matmul per
                    # pair computes blockdiag(M_2pr, M_2pr+1) plus junk
                    # off-diagonal blocks (discarded at evac). start=True zeroes
                    # the whole 2KB PSUM zero-region (= 4 pair blocks), so only
                    # the first pair per region starts the group and only the
                    # last stops it.
                    kn_flat = kn_bf.rearrange("p h f -> p (h f)")
                    for pr in range(IC):
                        nc.tensor.matmul(
                            M_ps[:, pr, :],
                            kn_flat[:, pr * P : (pr + 1) * P],
                            v_bf[:, pr * P : (pr + 1) * P],
                            start=(t == 0 and pr % 4 == 0),
                            stop=(t == NKT - 1 and pr % 4 == 3),
                            skip_group_check=True,
                        )
                for po in (0, 64):
                    nc.scalar.activation(
                        out=M_sb[po : po + 64, :, po : po + 64],
                        in_=M_ps[po : po + 64, :, po : po + 64],
                        func=AF.Copy,
                    )
            if DBG:
                with tc.tile_pool(name="dbgp", bufs=1) as dbgp:
                    mf = dbgp.tile([P, IC, DH], F32)
                    for po in (0, 64):
                        nc.vector.tensor_copy(
                            out=mf[po : po + 64, :, :],
                            in_=M_ps[po : po + 64, :, po : po + 64],
                        )
                    nc.sync.dma_start(dbg["dbg_M"].rearrange("p (c f) -> p c f", c=IC)[:], mf[:])

            # ---------------- Phase 2: Q -> attn -> x ----------------
            with (
                tc.tile_pool(name="q_io", bufs=3) as q_io,
                tc.tile_pool(name="q_mid", bufs=3) as q_mid,
                tc.tile_pool(name="q_stats", bufs=4) as q_stats,
                tc.tile_pool(name="q_ps", bufs=2, space="PSUM") as q_ps,
                tc.tile_pool(name="x_ps", bufs=2, space="PSUM") as x_ps,
                tc.tile_pool(name="at_ps", bufs=2, space="PSUM") as at_ps,
            ):
                for t in range(NQT):
                    q_in = q_io.tile([P, D], F32, tag="q_in")
                    nc.sync.dma_start(q_in[:], Qt[t][:])
                    rs, nmu = _ln_stats_ops(nc, q_stats, q_in, P, D, eps_tile)
                    qn_std = q_mid.tile([P, D], BF16, tag="q_std")
                    nc.scalar.activation(
                        out=qn_std[:], in_=q_in[:], func=AF.Identity, bias=nmu[:], scale=rs[:]
                    )
                    qnT = q_mid.tile([P, DC, P], BF16, tag="q_xnT")
                    for c in range(DC):
                        nc.sync.dma_start(
                            qnT[:, c, :], qn_std[:, c * P : (c + 1) * P], transpose=True
                        )
                    pss = []
                    for g in range(2):
                        ps = q_ps.tile([P, 512], F32, tag="q_proj")
                        pss.append(ps)
                    for c in range(DC):
                        for g in range(2):
                            nc.tensor.matmul(
                                pss[g][:],
                                qnT[:, c, :],
                                wq_sb[:, c, g * 512 : (g + 1) * 512],
                                start=(c == 0),
                                stop=(c == DC - 1) and ("bq" not in brow_sb),
                            )
                    for g in range(2):
                        if "bq" in brow_sb:
                            nc.tensor.matmul(
                                pss[g][:],
                                ones_row[:],
                                brow_sb["bq"][:, g * 512 : (g + 1) * 512],
                                start=False,
                                stop=True,
                                skip_group_check=True,
                            )
                    # l2-normalize q per head -> qn bf16
                    qn_bf = q_mid.tile([P, H, DH], BF16, tag="qn_bf")
                    for g in range(2):
                        sq = q_mid.tile([P, 512], F32, tag="q_sq")
                        nc.scalar.activation(out=sq[:], in_=pss[g][:], func=AF.Square)
                        ss = q_stats.tile([P, 8, 1], F32, tag="ql2_ss")
                        nc.vector.reduce_sum(
                            out=ss[:],
                            in_=sq.rearrange("p (h f) -> p h f", h=8),
                            axis=mybir.AxisListType.X,
                        )
                        rn = q_stats.tile([P, 8, 1], F32, tag="ql2_rn")
                        nc.scalar.activation(out=rn[:], in_=ss[:], func=AF.Sqrt)
                        nc.vector.tensor_scalar_max(out=rn[:], in0=rn[:], scalar1=1e-12)
                        nc.vector.reciprocal(out=rn[:], in_=rn[:])
                        nc.vector.tensor_tensor(
                            out=qn_bf[:, g * 8 : (g + 1) * 8, :],
                            in0=pss[g].rearrange("p (h f) -> p h f", h=8),
                            in1=rn.to_broadcast([P, 8, DH]),
                            op=ALU.mult,
                        )
                    # transpose qn -> [INNER, tok] feature-major
                    qn_flat = qn_bf.rearrange("p h f -> p (h f)")
                    qnT2 = q_mid.tile([P, IC, P], BF16, tag="qnT2")
                    for c in range(IC):
                        nc.sync.dma_start(
                            qnT2[:, c, :], qn_flat[:, c * P : (c + 1) * P], transpose=True
                        )
                    # attn^T[h] = M_h^T @ qn_h^T  -> [INNER, tok] chunks
                    a_ps = at_ps.tile([P, IC, P], F32, tag="attn_ps")
                    for pr in range(IC):
                        nc.tensor.matmul(
                            a_ps[:, pr, :],
                            M_sb[:, pr, :],
                            qnT2[:, pr, :],
                            start=True,
                            stop=True,
                            skip_group_check=True,
                        )
                    aT_bf = q_mid.tile([P, IC, P], BF16, tag="aT_bf")
                    nc.scalar.activation(out=aT_bf[:], in_=a_ps[:], func=AF.Copy)
                    if DBG and t == 0:
                        qnf = q_mid.tile([P, INNER], F32, tag="dbg_qnf")
                        nc.vector.tensor_copy(out=qnf.rearrange("p (h f) -> p h f", h=H), in_=qn_bf[:])
                        nc.sync.dma_start(dbg["dbg_qn"][:], qnf[:])
                        atf = q_mid.tile([P, IC, P], F32, tag="dbg_atf")
                        nc.vector.tensor_copy(out=atf[:], in_=a_ps[:])
                        nc.sync.dma_start(dbg["dbg_aT"].rearrange("p (c f) -> p c f", c=IC)[:], atf[:])
                    # x = Q + attn @ wo (+bo)
                    xps = []
                    for g in range(2):
                        ps = x_ps.tile([P, 512], F32, tag="x_proj")
                        xps.append(ps)
                    for c in range(IC):
                        for g in range(2):
                            nc.tensor.matmul(
                                xps[g][:],
                                aT_bf[:, c, :],
                                wo_sb[:, c, g * 512 : (g + 1) * 512],
                                start=(c == 0),
                                stop=(c == IC - 1) and ("bo" not in brow_sb),
                            )
                    for g in range(2):
                        if "bo" in brow_sb:
                            nc.tensor.matmul(
                                xps[g][:],
                                ones_row[:],
                                brow_sb["bo"][:, g * 512 : (g + 1) * 512],
                                start=False,
                                stop=True,
                                skip_group_check=True,
                            )
                        nc.vector.tensor_tensor(
                            out=x_sb[:, t, g * 512 : (g + 1) * 512],
                            in0=xps[g][:],
                            in1=q_in[:, g * 512 : (g + 1) * 512],
                            op=ALU.add,
                        )

            if DBG:
                nc.sync.dma_start(dbg["dbg_x"][:], x_sb[:, 0, :])
            # ---------------- Phase 3: FFN (groups of 4 token tiles) ----------------
            with (
                tc.tile_pool(name="f_mid", bufs=3) as f_mid,
                tc.tile_pool(name="f_w", bufs=3) as f_w,
                tc.tile_pool(name="f_h", bufs=1) as f_h,
                tc.tile_pool(name="f_stats", bufs=4) as f_stats,
                tc.tile_pool(name="f_out", bufs=3) as f_out,
                tc.tile_pool(name="h_ps", bufs=2, space="PSUM") as h_ps,
                tc.tile_pool(name="y_ps", bufs=4, space="PSUM") as y_ps,
            ):
                GT = 2  # token tiles per FFN group
                for grp in range(NQT // GT):
                    xnT4 = f_mid.tile([P, DC, GT * P], BF16, tag="xnT4")
                    for tt in range(GT):
                        t = grp * GT + tt
                        rs, nmu = _ln_stats_ops(
                            nc, f_stats, x_sb[:, t, :], P, D, eps_tile
                        )
                        xn = f_mid.tile([P, D], BF16, tag="f_std")
                        nc.scalar.activation(
                            out=xn[:],
                            in_=x_sb[:, t, :],
                            func=AF.Identity,
                            bias=nmu[:],
                            scale=rs[:],
                        )
                        for c in range(DC):
                            nc.sync.dma_start(
                                xnT4[:, c, tt * P : (tt + 1) * P],
                                xn[:, c * P : (c + 1) * P],
                                transpose=True,
                            )
                    # h^T = gelu(w1^T @ xn^T + b1) : feature-major [MLP, 4*128]
                    h4 = f_h.tile([P, MC, GT * P], BF16, tag="h4")
                    for m in range(MC):
                        w1t = f_w.tile([P, DC, P], BF16, tag="w1t")
                        nc.scalar.dma_start(w1t[:], w1_v[m])
                        hp = h_ps.tile([P, GT * P], F32, tag="h_ps_t")
                        for c in range(DC):
                            nc.tensor.matmul(
                                hp[:],
                                w1t[:, c, :],
                                xnT4[:, c, :],
                                start=(c == 0),
                                stop=(c == DC - 1),
                            )
                        nc.scalar.activation(
                            out=h4[:, m, :],
                            in_=hp[:],
                            func=AF.Gelu,
                            bias=bff1_sb[:, m : m + 1],
                            scale=1.0,
                        )
                    # y = x + h @ w2 (+ b2)
                    yps = [
                        [
                            y_ps.tile(
                                [P, 512], F32, tag="y_ps_t", name=f"yps_{tt}_{g}"
                            )
                            for g in range(2)
                        ]
                        for tt in range(GT)
                    ]
                    for m in range(MC):
                        w2t = f_w.tile([P, D], BF16, tag="w2t")
                        nc.gpsimd.dma_start(w2t[:], w2_v[:, m, :])
                        for tt in range(GT):
                            for g in range(2):
                                nc.tensor.matmul(
                                    yps[tt][g][:],
                                    h4[:, m, tt * P : (tt + 1) * P],
                                    w2t[:, g * 512 : (g + 1) * 512],
                                    start=(m == 0),
                                    stop=(m == MC - 1) and ("b2" not in brow_sb),
                                )
                    for tt in range(GT):
                        t = grp * GT + tt
                        for g in range(2):
                            if "b2" in brow_sb:
                                nc.tensor.matmul(
                                    yps[tt][g][:],
                                    ones_row[:],
                                    brow_sb["b2"][:, g * 512 : (g + 1) * 512],
                                    start=False,
                                    stop=True,
                                    skip_group_check=True,
                                )
                            y_out = f_out.tile([P, 512], F32, tag="y_out")
                            nc.vector.tensor_tensor(
                                out=y_out[:],
                                in0=yps[tt][g][:],
                                in1=x_sb[:, t, g * 512 : (g + 1) * 512],
                                op=ALU.add,
                            )
                            nc.sync.dma_start(
                                Yt[t][:, g * 512 : (g + 1) * 512], y_out[:]
                            )

    nc.compile()
    return nc


def prep_inputs(inputs):
    """Host-side shard + weight folding. Returns (in_maps, bias_rows)."""
    f32 = np.float32
    bf = ml_dtypes.bfloat16
    g1 = np.asarray(inputs["ln1_g"], f32)
    b1ln = np.asarray(inputs["ln1_b"], f32)
    g2 = np.asarray(inputs["ln2_g"], f32)
    b2ln = np.asarray(inputs["ln2_b"], f32)
    wq = np.asarray(inputs["wq"], f32)
    wk = np.asarray(inputs["wk"], f32)
    wv = np.asarray(inputs["wv"], f32)
    wo = np.asarray(inputs["wo"], f32)
    w1 = np.asarray(inputs["w1"], f32)
    w2 = np.asarray(inputs["w2"], f32)

    bias_rows = {
        "bq": (b1ln @ wq).astype(f32),
        "bk": (b1ln @ wk).astype(f32),
        "bv": (b1ln @ wv).astype(f32),
        "bo": np.asarray(inputs["bo"], f32),
        "b2": np.asarray(inputs["b2"], f32),
    }
    bff1 = (b2ln @ w1 + np.asarray(inputs["b1"], f32)).astype(f32)
    bff1_tile = np.ascontiguousarray(bff1.reshape(MC, P).T)  # [P, MC]

    wq_b = np.ascontiguousarray((g1[:, None] * wq).astype(bf))
    wk_b = np.ascontiguousarray((g1[:, None] * wk).astype(bf))
    wv_b = np.ascontiguousarray((g1[:, None] * wv).astype(bf))
    wo_b = np.ascontiguousarray(wo.astype(bf))
    w1g = (g2[:, None] * w1).astype(bf)
    # pack w1 so each streamed [P, DC*P] tile is one contiguous block:
    # packed[m, p, c, q] = w1g[c*128+p, m*128+q]
    w1_b = np.ascontiguousarray(
        w1g.reshape(DC, P, MC, P).transpose(2, 1, 0, 3).reshape(MC * P, DC * P)
    )
    w2_b = np.ascontiguousarray(w2.astype(bf))

    Q = np.asarray(inputs["Q"], f32)
    K = np.asarray(inputs["K"], f32)
    V = np.asarray(inputs["V"], f32)

    in_maps = []
    for c in range(NCORES):
        b = c // 2
        r0 = (c % 2) * TQ
        m = {
            "q_tok": np.ascontiguousarray(Q[b, r0 : r0 + TQ]),
            "k_tok": np.ascontiguousarray(K[b].astype(bf)),
            "v_tok": np.ascontiguousarray(V[b].astype(bf)),
            "wq": wq_b,
            "wk": wk_b,
            "wv": wv_b,
            "wo": wo_b,
            "w1": w1_b,
            "w2": w2_b,
            "bff1": bff1_tile,
        }
        for name, row in bias_rows.items():
            if np.any(row):
                m["brow_" + name] = row[None, :].astype(bf)
        in_maps.append(m)
    return in_maps, bias_rows


_NC_CACHE = {}


def get_nc(bias_key):
    if bias_key not in _NC_CACHE:
        # bias_key is a tuple of names with nonzero rows; build needs the rows
        # only for zero-checks, so reconstruct flags
        raise KeyError
    return _NC_CACHE[bias_key]


def kernel(**inputs) -> np.ndarray:
    from concourse.bass_utils import run_bass_kernel_spmd

    in_maps, bias_rows = prep_inputs(inputs)
    bias_key = tuple(sorted(n for n, r in bias_rows.items() if np.any(r)))
    if bias_key not in _NC_CACHE:
        _NC_CACHE[bias_key] = build_nc(bias_rows)
    nc = _NC_CACHE[bias_key]
    res = run_bass_kernel_spmd(nc, in_maps, core_ids=list(range(NCORES)))
    out = np.empty((B, N, D), np.float32)
    for c in range(NCORES):
        b = c // 2
        r0 = (c % 2) * TQ
        out[b, r0 : r0 + TQ] = res.results[c]["y"]
    return out



# revision 10
# speedup vs baseline: 1.2106x; 1.2106x over previous
"""CosineTransformerBlock Trainium2 kernel (8 NeuronCores, SPMD).

Sharding: core c handles batch b = c // 2.  Query rows AND key/value rows
[(c % 2) * 1024 : (c % 2) * 1024 + 1024] of that batch.  The per-head
attention matrices M_h = sum_k kn_k^T v_k are computed from each core's
KV half and pair-AllReduced (256KB) -- this halves the K/V projection
work vs. duplicating it.

Key algebraic transform: cosine attention has no softmax, so
    (qn @ kn^T) @ v  ==  qn @ (kn^T @ v)
which turns the O(N^2) attention into two tiny per-head [64,64] matmuls.

Precision: q/k/v/wo/attention matmuls in bf16 (fp8 there fails the 2e-2
gate); the FFN runs in fp8 e4m3 with DoubleRow perf mode (two 128-row
contraction chunks per instruction).  Scale folding (all powers of 2):
  - w1 scaled by s1, xn by sx -> folded out in the gelu-evac scale;
  - w2, wo, Q, bo, b2 scaled by s2 -> the whole residual stream x and the
    output y are carried as s2*x; the host divides the result by s2.
LN affine (g, b) is folded into the following weight matrix on the host.
"""

import os
import sys

sys.path.insert(0, "/opt/trn_rl_repo")

import numpy as np
import ml_dtypes

# ---- problem shapes (hardcoded per contract) ----
B, N, D = 4, 2048, 1024
H, DH = 16, 64
INNER = H * DH  # 1024
MLP = 4096
EPS = 1e-5
NCORES = 8
TQ = N // 2  # 1024 query tokens per core
TKV = N // 2  # 1024 kv tokens per core (pair-split + M all-reduce)
P = 128
DC = D // P  # 8 chunks of the model dim
IC = INNER // P  # 8
MC = MLP // P  # 32
NQT = TQ // P  # 8 q token tiles
NKT = TKV // P  # 8 kv token tiles
SX = 32.0  # fp8 scale on the FFN ln output

BF16 = None
F32 = None
FP8 = None


def _dt():
    global BF16, F32, FP8
    import concourse.mybir as mybir

    BF16 = mybir.dt.bfloat16
    F32 = mybir.dt.float32
    FP8 = mybir.dt.float8e4
    return mybir


def _ln_stats_ops(nc, pool, x_tile, dfree, eps_tile, sqrt_scale):
    """bn_stats/bn_aggr over free dim.

    Returns (rs, nmu): rs = k/sqrt(var+eps), nmu = -mu*rs, where
    k = 1/sqrt(sqrt_scale) is folded in via the Sqrt activation scale
    (rs = 1/sqrt(sqrt_scale*var + eps_tile))."""
    import concourse.mybir as mybir

    nsub = (dfree + 511) // 512
    stats = pool.tile([P, nsub, 6], F32, tag="ln_stats")
    xv = x_tile.rearrange("p (s f) -> p s f", s=nsub)
    for s in range(nsub):
        nc.vector.bn_stats(out=stats[:, s, :], in_=xv[:, s, :])
    mv = pool.tile([P, 2], F32, tag="ln_mv")
    nc.vector.bn_aggr(out=mv[:], in_=stats[:])
    rs = pool.tile([P, 1], F32, tag="ln_rs")
    nc.scalar.activation(
        out=rs[:],
        in_=mv[:, 1:2],
        func=mybir.ActivationFunctionType.Sqrt,
        bias=eps_tile[:],
        scale=sqrt_scale,
    )
    nc.vector.reciprocal(out=rs[:], in_=rs[:])
    nmu = pool.tile([P, 1], F32, tag="ln_nmu")
    nc.vector.tensor_scalar(
        out=nmu[:],
        in0=mv[:, 0:1],
        scalar1=rs[:],
        scalar2=-1.0,
        op0=mybir.AluOpType.mult,
        op1=mybir.AluOpType.mult,
    )
    return rs, nmu


def _l2_ops(nc, mid, stats, pss, out_bf, mybir):
    """Per-head l2 normalization of a [P, 1024] PSUM pair into out_bf."""
    AF = mybir.ActivationFunctionType
    ALU = mybir.AluOpType
    for g in range(2):
        sq = mid.tile([P, 512], F32, tag="l2_sq")
        nc.scalar.activation(out=sq[:], in_=pss[g][:], func=AF.Square)
        ss = stats.tile([P, 8, 1], F32, tag="l2_ss")
        nc.vector.reduce_sum(
            out=ss[:],
            in_=sq.rearrange("p (h f) -> p h f", h=8),
            axis=mybir.AxisListType.X,
        )
        rn = stats.tile([P, 8, 1], F32, tag="l2_rn")
        nc.scalar.activation(out=rn[:], in_=ss[:], func=AF.Sqrt)
        nc.vector.tensor_scalar_max(out=rn[:], in0=rn[:], scalar1=1e-12)
        nc.vector.reciprocal(out=rn[:], in_=rn[:])
        nc.vector.tensor_tensor(
            out=out_bf[:, g * 8 : (g + 1) * 8, :],
            in0=pss[g].rearrange("p (h f) -> p h f", h=8),
            in1=rn.to_broadcast([P, 8, DH]),
            op=ALU.mult,
        )


def build_nc(bias_rows, s2, gelu_scale):
    mybir = _dt()
    import concourse.bass as bass
    import concourse.tile as tile
    from concourse import bacc

    AF = mybir.ActivationFunctionType
    ALU = mybir.AluOpType

    nc = bacc.Bacc("TRN2", target_bir_lowering=False, debug=False, num_devices=NCORES)

    # ---- DRAM I/O ----
    Qd = nc.dram_tensor("q_tok", [TQ, D], F32, kind="ExternalInput").ap()
    Kd = nc.dram_tensor("k_tok", [TKV, D], BF16, kind="ExternalInput").ap()
    Vd = nc.dram_tensor("v_tok", [TKV, D], BF16, kind="ExternalInput").ap()
    wq_d = nc.dram_tensor("wq", [D, INNER], BF16, kind="ExternalInput").ap()
    wk_d = nc.dram_tensor("wk", [D, INNER], BF16, kind="ExternalInput").ap()
    wv_d = nc.dram_tensor("wv", [D, INNER], BF16, kind="ExternalInput").ap()
    wo_d = nc.dram_tensor("wo", [INNER, D], BF16, kind="ExternalInput").ap()
    w1_d = nc.dram_tensor("w1", [P, MC * DC * P], FP8, kind="ExternalInput").ap()
    w2_d = nc.dram_tensor("w2", [P, (MC // 2) * 2 * D], FP8, kind="ExternalInput").ap()
    bff1_d = None
    if np.any(bias_rows["bff1"]):
        bff1_d = nc.dram_tensor("bff1", [P, MC], F32, kind="ExternalInput").ap()
    brow_d = {}
    for name in ("bq", "bk", "bv", "bo", "b2"):
        if np.any(bias_rows[name]):
            brow_d[name] = nc.dram_tensor(
                "brow_" + name, [1, bias_rows[name].shape[0]], BF16,
                kind="ExternalInput",
            ).ap()
    m_dram = nc.dram_tensor("m_ar", [P, IC * DH], F32).ap()
    Yd = nc.dram_tensor("y", [TQ, D], F32, kind="ExternalOutput").ap()

    Qt = Qd.rearrange("(t p) d -> t p d", p=P)
    Kt = Kd.rearrange("(t p) d -> t p d", p=P)
    Vt = Vd.rearrange("(t p) d -> t p d", p=P)
    Yt = Yd.rearrange("(t p) d -> t p d", p=P)
    wq_v = wq_d.rearrange("(c p) n -> p c n", p=P)
    wk_v = wk_d.rearrange("(c p) n -> p c n", p=P)
    wv_v = wv_d.rearrange("(c p) n -> p c n", p=P)
    wo_v = wo_d.rearrange("(c p) n -> p c n", p=P)

    with tile.TileContext(nc) as tc:
        with tc.tile_pool(name="singles", bufs=1) as singles:
            # ---- resident state ----
            wq_sb = singles.tile([P, DC, INNER], BF16)
            q_in = singles.tile([P, NQT, D], F32)
            xnT4 = singles.tile([P, DC, TQ], FP8)
            eps_tile = singles.tile([P, 1], F32)
            nc.vector.memset(eps_tile[:], EPS)
            # ffn ln runs on s2-scaled x with sx fold: rs = sx/sqrt(var+s2^2*eps)
            epsf_tile = singles.tile([P, 1], F32)
            nc.vector.memset(epsf_tile[:], EPS * s2 * s2 / (SX * SX))
            ones_row = singles.tile([1, P], BF16)
            nc.vector.memset(ones_row[:], 1.0)
            brow_sb = {}
            for name, ap in brow_d.items():
                t = singles.tile([1, ap.shape[1]], BF16, tag="brow_" + name)
                nc.sync.dma_start(t[:], ap[:])
                brow_sb[name] = t
            bff1_sb = None
            if bff1_d is not None:
                bff1_sb = singles.tile([P, MC], F32)
                nc.sync.dma_start(bff1_sb[:], bff1_d[:])
            M_sb = singles.tile([P, IC, P], BF16)
            nc.vector.memset(M_sb[:], 0.0)
            m_sb = singles.tile([P, IC, DH], F32)
            mr_sb = singles.tile([P, IC, DH], F32)

            # early bulk loads (gpsimd SWDGE: cheap issue)
            for t in range(NQT):
                nc.gpsimd.dma_start(q_in[:, t, :], Qt[t][:])
            nc.gpsimd.dma_start(wq_sb[:], wq_v[:])

            def bias_mm(ps, name, lo, hi):
                if name in brow_sb:
                    nc.tensor.matmul(
                        ps,
                        ones_row[:, : ps.shape[0]],
                        brow_sb[name][:, lo:hi],
                        start=False,
                        stop=True,
                        skip_group_check=True,
                    )
                    return True
                return False

            # ---------------- Phase 1: KV half -> partial M ----------------
            with (
                tc.tile_pool(name="kv_w", bufs=1) as kv_w,
                tc.tile_pool(name="kv_io", bufs=3) as kv_io,
                tc.tile_pool(name="kv_mid", bufs=3) as kv_mid,
                tc.tile_pool(name="kv_stats", bufs=4) as kv_stats,
                tc.tile_pool(name="kv_ps", bufs=4, space="PSUM") as kv_ps,
                tc.tile_pool(name="m_ps", bufs=1, space="PSUM") as m_ps_pool,
            ):
                wk_sb = kv_w.tile([P, DC, INNER], BF16)
                wv_sb = kv_w.tile([P, DC, INNER], BF16)
                for c in range(DC):
                    nc.sync.dma_start(wk_sb[:, c, :], wk_v[:, c, :])
                nc.gpsimd.dma_start(wv_sb[:], wv_v[:])
                M_ps = m_ps_pool.tile([P, IC, P], F32)
                for t in range(NKT):
                    kn_bf = None
                    v_bf = None
                    for which in ("k", "v"):
                        src = Kt[t] if which == "k" else Vt[t]
                        w_sb = wk_sb if which == "k" else wv_sb
                        bname = "bk" if which == "k" else "bv"
                        x_in = kv_io.tile([P, D], BF16, tag="kv_in")
                        nc.sync.dma_start(x_in[:], src[:])
                        rs, nmu = _ln_stats_ops(nc, kv_stats, x_in, D, eps_tile, 1.0)
                        xn = kv_mid.tile([P, D], BF16, tag="kv_std")
                        nc.scalar.activation(
                            out=xn[:], in_=x_in[:], func=AF.Identity, bias=nmu[:], scale=rs[:]
                        )
                        xnT = kv_mid.tile([P, DC, P], BF16, tag="kv_xnT")
                        for c in range(DC):
                            nc.sync.dma_start(
                                xnT[:, c, :], xn[:, c * P : (c + 1) * P], transpose=True
                            )
                        pss = [kv_ps.tile([P, 512], F32, tag="kv_proj", name=f"kvp{t}_{which}_{g}") for g in range(2)]
                        for c in range(DC):
                            for g in range(2):
                                nc.tensor.matmul(
                                    pss[g][:],
                                    xnT[:, c, :],
                                    w_sb[:, c, g * 512 : (g + 1) * 512],
                                    start=(c == 0),
                                    stop=(c == DC - 1) and (bname not in brow_sb),
                                )
                        for g in range(2):
                            bias_mm(pss[g][:], bname, g * 512, (g + 1) * 512)
                        if which == "v":
                            v_bf = kv_mid.tile([P, INNER], BF16, tag="v_bf")
                            for g in range(2):
                                nc.scalar.activation(
                                    out=v_bf[:, g * 512 : (g + 1) * 512],
                                    in_=pss[g][:],
                                    func=AF.Copy,
                                )
                        else:
                            kn_bf = kv_mid.tile([P, H, DH], BF16, tag="kn_bf")
                            _l2_ops(nc, kv_mid, kv_stats, pss, kn_bf, mybir)
                    kn_flat = kn_bf.rearrange("p h f -> p (h f)")
                    for pr in range(IC):
                        nc.tensor.matmul(
                            M_ps[:, pr, :],
                            kn_flat[:, pr * P : (pr + 1) * P],
                            v_bf[:, pr * P : (pr + 1) * P],
                            start=(t == 0 and pr % 4 == 0),
                            stop=(t == NKT - 1 and pr % 4 == 3),
                            skip_group_check=True,
                        )
                # evac the per-head diag blocks: head pair pr holds
                # M_2pr at [0:64, pr, 0:64] and M_2pr+1 at [64:128, pr, 64:128]
                for po in (0, 64):
                    nc.scalar.activation(
                        out=m_sb[po : po + 64, :, :],
                        in_=M_ps[po : po + 64, :, po : po + 64],
                        func=AF.Copy,
                    )
            # pair all-reduce of the partial M
            nc.sync.dma_start(m_dram.rearrange("p (c f) -> p c f", c=IC)[:], m_sb[:])
            nc.gpsimd.collective_compute(
                "AllReduce",
                ALU.add,
                replica_groups=[[0, 1], [2, 3], [4, 5], [6, 7]],
                ins=[m_dram[:]],
                outs=[m_dram[:]],
            )
            nc.sync.dma_start(mr_sb[:], m_dram.rearrange("p (c f) -> p c f", c=IC)[:])
            for po in (0, 64):
                nc.vector.tensor_copy(
                    out=M_sb[po : po + 64, :, po : po + 64],
                    in_=mr_sb[po : po + 64, :, :],
                )

            # ---------------- Phase 2+3: Q -> attn -> x (+ FFN prep) --------
            with tc.tile_pool(name="wide", bufs=1) as wide:
                w2_sb = wide.tile([P, MC // 2, 2, D], FP8)
                w2f = w2_sb.rearrange("p a b c -> p (a b c)")
                CH = 4096
                for i in range((MC // 2) * 2 * D // CH):
                    nc.gpsimd.dma_start(
                        w2f[:, i * CH : (i + 1) * CH], w2_d[:, i * CH : (i + 1) * CH]
                    )

                with (
                    tc.tile_pool(name="q_w", bufs=1) as q_w,
                    tc.tile_pool(name="q_mid", bufs=3) as q_mid,
                    tc.tile_pool(name="q_stats", bufs=4) as q_stats,
                    tc.tile_pool(name="q_ps", bufs=4, space="PSUM") as q_ps,
                    tc.tile_pool(name="at_ps", bufs=1, space="PSUM") as at_ps,
                    tc.tile_pool(name="x_ps", bufs=2, space="PSUM") as x_ps,
                ):
                    wo_sb = q_w.tile([P, IC, D], BF16)
                    nc.gpsimd.dma_start(wo_sb[:], wo_v[:])
                    for t in range(NQT):
                        rs, nmu = _ln_stats_ops(
                            nc, q_stats, q_in[:, t, :], D, eps_tile, 1.0
                        )
                        qx = q_mid.tile([P, D], BF16, tag="q_std")
                        nc.scalar.activation(
                            out=qx[:], in_=q_in[:, t, :], func=AF.Identity,
                            bias=nmu[:], scale=rs[:],
                        )
                        qxT = q_mid.tile([P, DC, P], BF16, tag="q_xnT")
                        for c in range(DC):
                            nc.sync.dma_start(
                                qxT[:, c, :], qx[:, c * P : (c + 1) * P], transpose=True
                            )
                        pss = [q_ps.tile([P, 512], F32, tag="q_proj", name=f"qp{t}_{g}") for g in range(2)]
                        for c in range(DC):
                            for g in range(2):
                                nc.tensor.matmul(
                                    pss[g][:],
                                    qxT[:, c, :],
                                    wq_sb[:, c, g * 512 : (g + 1) * 512],
                                    start=(c == 0),
                                    stop=(c == DC - 1) and ("bq" not in brow_sb),
                                )
                        for g in range(2):
                            bias_mm(pss[g][:], "bq", g * 512, (g + 1) * 512)
                        qn_bf = q_mid.tile([P, H, DH], BF16, tag="qn_bf")
                        _l2_ops(nc, q_mid, q_stats, pss, qn_bf, mybir)
                        qn_flat = qn_bf.rearrange("p h f -> p (h f)")
                        qnT2 = q_mid.tile([P, IC, P], BF16, tag="qnT2")
                        for c in range(IC):
                            nc.sync.dma_start(
                                qnT2[:, c, :], qn_flat[:, c * P : (c + 1) * P],
                                transpose=True,
                            )
                        a_ps = at_ps.tile([P, IC, P], F32, tag="attn_ps")
                        for pr in range(IC):
                            nc.tensor.matmul(
                                a_ps[:, pr, :],
                                M_sb[:, pr, :],
                                qnT2[:, pr, :],
                                start=True,
                                stop=True,
                                skip_group_check=True,
                            )
                        aT_bf = q_mid.tile([P, IC, P], BF16, tag="aT_bf")
                        nc.scalar.activation(out=aT_bf[:], in_=a_ps[:], func=AF.Copy)
                        xps = [x_ps.tile([P, 512], F32, tag="x_proj", name=f"xp{t}_{g}") for g in range(2)]
                        for c in range(IC):
                            for g in range(2):
                                nc.tensor.matmul(
                                    xps[g][:],
                                    aT_bf[:, c, :],
                                    wo_sb[:, c, g * 512 : (g + 1) * 512],
                                    start=(c == 0),
                                    stop=(c == IC - 1) and ("bo" not in brow_sb),
                                )
                        for g in range(2):
                            bias_mm(xps[g][:], "bo", g * 512, (g + 1) * 512)
                            nc.vector.tensor_tensor(
                                out=q_in[:, t, g * 512 : (g + 1) * 512],
                                in0=xps[g][:],
                                in1=q_in[:, t, g * 512 : (g + 1) * 512],
                                op=ALU.add,
                            )
                        # FFN prep for this tile: ln2 + transpose + fp8 cast
                        rs2, nmu2 = _ln_stats_ops(
                            nc, q_stats, q_in[:, t, :], D, epsf_tile, 1.0 / (SX * SX)
                        )
                        fx = q_mid.tile([P, D], BF16, tag="f_std")
                        nc.scalar.activation(
                            out=fx[:], in_=q_in[:, t, :], func=AF.Identity,
                            bias=nmu2[:], scale=rs2[:],
                        )
                        fxT = q_mid.tile([P, DC, P], BF16, tag="f_xnT")
                        for c in range(DC):
                            nc.sync.dma_start(
                                fxT[:, c, :], fx[:, c * P : (c + 1) * P], transpose=True
                            )
                        nc.vector.tensor_copy(
                            out=xnT4[:, :, t * P : (t + 1) * P], in_=fxT[:]
                        )

                # ---------------- Phase 4: FFN (fp8 DoubleRow) ----------------
                with (
                    tc.tile_pool(name="f_h", bufs=1) as f_h,
                    tc.tile_pool(name="f_w", bufs=4) as f_w,
                    tc.tile_pool(name="f_out", bufs=3) as f_out,
                ):
                    h4 = f_h.tile([P, MC, TQ], FP8)
                    MSTR = 2 * (DC // 2) * 2 * P  # dram columns per m-pair
                    with tc.tile_pool(name="h_ps", bufs=2, space="PSUM") as h_ps:
                      for mp in range(MC // 2):
                        w1t = f_w.tile([P, 2, DC // 2, 2, P], FP8, tag="w1t")
                        nc.gpsimd.dma_start(
                            w1t.rearrange("p a b c d -> p (a b c d)")[:],
                            w1_d[:, mp * MSTR : (mp + 1) * MSTR],
                        )
                        hp2 = h_ps.tile([P, 2, TQ], F32, tag="h_ps_t")
                        for jm in range(2):
                            m = 2 * mp + jm
                            for hh in range(2):
                                hsl = slice(hh * 512, (hh + 1) * 512)
                                for c2 in range(DC // 2):
                                    nc.tensor.matmul(
                                        hp2[:, jm, hsl],
                                        w1t[:, jm, c2, :, :],
                                        xnT4[:, 2 * c2 : 2 * c2 + 2, hsl],
                                        start=(c2 == 0),
                                        stop=(c2 == DC // 2 - 1),
                                        perf_mode=mybir.MatmulPerfMode.DoubleRow,
                                    )
                        if bff1_sb is not None:
                            for jm in range(2):
                                m = 2 * mp + jm
                                nc.scalar.activation(
                                    out=h4[:, m, :],
                                    in_=hp2[:, jm, :],
                                    func=AF.Gelu,
                                    bias=bff1_sb[:, m : m + 1],
                                    scale=gelu_scale,
                                )
                        else:
                            nc.scalar.activation(
                                out=h4[:, 2 * mp : 2 * mp + 2, :],
                                in_=hp2[:],
                                func=AF.Gelu,
                                bias=0.0,
                                scale=gelu_scale,
                            )
                    with tc.tile_pool(name="y_ps", bufs=4, space="PSUM") as y_ps:
                      for t in range(NQT):
                        for g in range(2):
                            yp = y_ps.tile([P, 512], F32, tag="y_ps_t")
                            for m2 in range(MC // 2):
                                nc.tensor.matmul(
                                    yp[:],
                                    h4[:, 2 * m2 : 2 * m2 + 2, t * P : (t + 1) * P],
                                    w2_sb[:, m2, :, g * 512 : (g + 1) * 512],
                                    start=(m2 == 0),
                                    stop=(m2 == MC // 2 - 1) and ("b2" not in brow_sb),
                                    perf_mode=mybir.MatmulPerfMode.DoubleRow,
                                )
                            bias_mm(yp[:], "b2", g * 512, (g + 1) * 512)
                            y_out = f_out.tile([P, 512], F32, tag="y_out")
                            nc.vector.tensor_tensor(
                                out=y_out[:],
                                in0=yp[:],
                                in1=q_in[:, t, g * 512 : (g + 1) * 512],
                                op=ALU.add,
                            )
                            nc.scalar.dma_start(
                                Yt[t][:, g * 512 : (g + 1) * 512], y_out[:]
                            )

    nc.compile()
    return nc


def _pow2_scale(arr, target=224.0):
    m = float(np.abs(arr).max())
    if m == 0:
        return 1.0
    return float(2.0 ** np.floor(np.log2(target / m)))


def prep_inputs(inputs):
    """Host-side shard + weight folding. Returns (in_maps, bias_rows, s1, s2)."""
    f32 = np.float32
    bf = ml_dtypes.bfloat16
    f8 = ml_dtypes.float8_e4m3
    g1 = np.asarray(inputs["ln1_g"], f32)
    b1ln = np.asarray(inputs["ln1_b"], f32)
    g2 = np.asarray(inputs["ln2_g"], f32)
    b2ln = np.asarray(inputs["ln2_b"], f32)
    wq = np.asarray(inputs["wq"], f32)
    wk = np.asarray(inputs["wk"], f32)
    wv = np.asarray(inputs["wv"], f32)
    wo = np.asarray(inputs["wo"], f32)
    w1 = np.asarray(inputs["w1"], f32)
    w2 = np.asarray(inputs["w2"], f32)

    w1g = g2[:, None] * w1
    s1 = _pow2_scale(w1g)
    s2 = _pow2_scale(w2)

    bias_rows = {
        "bq": (b1ln @ wq).astype(f32),
        "bk": (b1ln @ wk).astype(f32),
        "bv": (b1ln @ wv).astype(f32),
        "bo": (np.asarray(inputs["bo"], f32) * s2),
        "b2": (np.asarray(inputs["b2"], f32) * s2),
        "bff1": (b2ln @ w1 + np.asarray(inputs["b1"], f32)).astype(f32),
    }
    bff1_tile = np.ascontiguousarray(bias_rows["bff1"].reshape(MC, P).T)

    wq_b = np.ascontiguousarray((g1[:, None] * wq).astype(bf))
    wk_b = np.ascontiguousarray((g1[:, None] * wk).astype(bf))
    wv_b = np.ascontiguousarray((g1[:, None] * wv).astype(bf))
    wo_b = np.ascontiguousarray((wo * s2).astype(bf))
    # w1 packed for DoubleRow: [p, m, c2, i, q] = (s1*w1g)[(2c2+i)*128+p, m*128+q]
    w1s = (w1g * s1).astype(f8).astype(f32)
    w1p = (
        w1s.reshape(DC // 2, 2, P, MC, P)  # [c2, i, p, m, q]
        .transpose(2, 3, 0, 1, 4)  # [p, m, c2, i, q]
        .reshape(P, MC * DC * P)
    )
    w1_b = np.ascontiguousarray(w1p.astype(f8))
    # w2 packed for DoubleRow: [p, m2, i, of] = (s2*w2)[(2m2+i)*128+p, of]
    w2s = (w2 * s2).astype(f8).astype(f32)
    w2p = (
        w2s.reshape(MC // 2, 2, P, D)  # [m2, i, p, of]
        .transpose(2, 0, 1, 3)  # [p, m2, i, of]
        .reshape(P, (MC // 2) * 2 * D)
    )
    w2_b = np.ascontiguousarray(w2p.astype(f8))

    Q = np.asarray(inputs["Q"], f32) * s2
    K = np.asarray(inputs["K"], f32)
    V = np.asarray(inputs["V"], f32)

    in_maps = []
    for c in range(NCORES):
        b = c // 2
        r0 = (c % 2) * TQ
        m = {
            "q_tok": np.ascontiguousarray(Q[b, r0 : r0 + TQ]),
            "k_tok": np.ascontiguousarray(K[b, r0 : r0 + TKV].astype(bf)),
            "v_tok": np.ascontiguousarray(V[b, r0 : r0 + TKV].astype(bf)),
            "wq": wq_b,
            "wk": wk_b,
            "wv": wv_b,
            "wo": wo_b,
            "w1": w1_b,
            "w2": w2_b,
        }
        if np.any(bias_rows["bff1"]):
            m["bff1"] = bff1_tile
        for name in ("bq", "bk", "bv", "bo", "b2"):
            if np.any(bias_rows[name]):
                m["brow_" + name] = bias_rows[name][None, :].astype(bf)
        in_maps.append(m)
    return in_maps, bias_rows, s1, s2


def gather(results, s2):
    out = np.empty((B, N, D), np.float32)
    inv = 1.0 / s2
    for c in range(NCORES):
        b = c // 2
        r0 = (c % 2) * TQ
        out[b, r0 : r0 + TQ] = results[c]["y"] * inv
    return out


_NC_CACHE = {}


def get_nc(bias_rows, s1, s2):
    key = (tuple(sorted(n for n, r in bias_rows.items() if np.any(r))), s1, s2)
    if key not in _NC_CACHE:
        _NC_CACHE[key] = build_nc(bias_rows, s2, 1.0 / (s1 * SX))
    return _NC_CACHE[key]


def kernel(**inputs) -> np.ndarray:
    from concourse.bass_utils import run_bass_kernel_spmd

    in_maps, bias_rows, s1, s2 = prep_inputs(inputs)
    nc = get_nc(bias_rows, s1, s2)
    res = run_bass_kernel_spmd(nc, in_maps, core_ids=list(range(NCORES)))
    return gather(res.results, s2)


# revision 12
# speedup vs baseline: 1.6790x; 1.3869x over previous
"""CosineTransformerBlock Trainium2 kernel (8 NeuronCores, SPMD).

Sharding: core c handles batch b = c // 2.  Query rows AND key/value rows
[(c % 2) * 1024 : (c % 2) * 1024 + 1024] of that batch.  The per-head
attention matrices M_h = sum_k kn_k^T v_k are computed from each core's
KV half and pair-AllReduced (256KB) -- this halves the K/V projection
work vs. duplicating it.

Key algebraic transform: cosine attention has no softmax, so
    (qn @ kn^T) @ v  ==  qn @ (kn^T @ v)
which turns the O(N^2) attention into two tiny per-head [64,64] matmuls.

Precision: q/k/v/wo/attention matmuls in bf16 (fp8 there fails the 2e-2
gate); the FFN runs in fp8 e4m3 with DoubleRow perf mode (two 128-row
contraction chunks per instruction).  Scale folding (all powers of 2):
  - w1 scaled by s1, xn by sx -> folded out in the gelu-evac scale;
  - w2, wo, Q, bo, b2 scaled by s2 -> the whole residual stream x and the
    output y are carried as s2*x; the host divides the result by s2.
LN affine (g, b) is folded into the following weight matrix on the host.
"""

import os
import sys

sys.path.insert(0, "/opt/trn_rl_repo")

import numpy as np
import ml_dtypes

# ---- problem shapes (hardcoded per contract) ----
B, N, D = 4, 2048, 1024
H, DH = 16, 64
INNER = H * DH  # 1024
MLP = 4096
EPS = 1e-5
NCORES = 8
TQ = N // 2  # 1024 query tokens per core
TKV = N // 2  # 1024 kv tokens per core (pair-split + M all-reduce)
P = 128
DC = D // P  # 8 chunks of the model dim
IC = INNER // P  # 8
MC = MLP // P  # 32
NQT = TQ // P  # 8 q token tiles
NKT = TKV // P  # 8 kv token tiles
SX = 32.0  # fp8 scale on the FFN ln output

BF16 = None
F32 = None
FP8 = None


def _dt():
    global BF16, F32, FP8
    import concourse.mybir as mybir

    BF16 = mybir.dt.bfloat16
    F32 = mybir.dt.float32
    FP8 = mybir.dt.float8e4
    return mybir


def _ln_stats_ops(nc, pool, x_tile, dfree, eps_tile, sqrt_scale):
    """bn_stats/bn_aggr over free dim.

    Returns (rs, nmu): rs = k/sqrt(var+eps), nmu = -mu*rs, where
    k = 1/sqrt(sqrt_scale) is folded in via the Sqrt activation scale
    (rs = 1/sqrt(sqrt_scale*var + eps_tile))."""
    import concourse.mybir as mybir

    nsub = (dfree + 511) // 512
    stats = pool.tile([P, nsub, 6], F32, tag="ln_stats")
    xv = x_tile.rearrange("p (s f) -> p s f", s=nsub)
    for s in range(nsub):
        nc.vector.bn_stats(out=stats[:, s, :], in_=xv[:, s, :])
    mv = pool.tile([P, 2], F32, tag="ln_mv")
    nc.vector.bn_aggr(out=mv[:], in_=stats[:])
    rs = pool.tile([P, 1], F32, tag="ln_rs")
    nc.scalar.activation(
        out=rs[:],
        in_=mv[:, 1:2],
        func=mybir.ActivationFunctionType.Sqrt,
        bias=eps_tile[:],
        scale=sqrt_scale,
    )
    nc.vector.reciprocal(out=rs[:], in_=rs[:])
    nmu = pool.tile([P, 1], F32, tag="ln_nmu")
    nc.vector.tensor_scalar(
        out=nmu[:],
        in0=mv[:, 0:1],
        scalar1=rs[:],
        scalar2=-1.0,
        op0=mybir.AluOpType.mult,
        op1=mybir.AluOpType.mult,
    )
    return rs, nmu


def _l2_ops(nc, mid, stats, pss, out_bf, mybir):
    """Per-head l2 normalization of a [P, 1024] PSUM pair into out_bf."""
    AF = mybir.ActivationFunctionType
    ALU = mybir.AluOpType
    for g in range(2):
        sq = mid.tile([P, 512], F32, tag="l2_sq")
        nc.scalar.activation(out=sq[:], in_=pss[g][:], func=AF.Square)
        ss = stats.tile([P, 8, 1], F32, tag="l2_ss")
        nc.vector.reduce_sum(
            out=ss[:],
            in_=sq.rearrange("p (h f) -> p h f", h=8),
            axis=mybir.AxisListType.X,
        )
        rn = stats.tile([P, 8, 1], F32, tag="l2_rn")
        nc.scalar.activation(out=rn[:], in_=ss[:], func=AF.Sqrt)
        nc.vector.tensor_scalar_max(out=rn[:], in0=rn[:], scalar1=1e-12)
        nc.vector.reciprocal(out=rn[:], in_=rn[:])
        nc.vector.tensor_tensor(
            out=out_bf[:, g * 8 : (g + 1) * 8, :],
            in0=pss[g].rearrange("p (h f) -> p h f", h=8),
            in1=rn.to_broadcast([P, 8, DH]),
            op=ALU.mult,
        )


def build_nc(bias_rows, s2, gelu_scale):
    mybir = _dt()
    import concourse.bass as bass
    import concourse.tile as tile
    from concourse import bacc

    AF = mybir.ActivationFunctionType
    ALU = mybir.AluOpType

    nc = bacc.Bacc("TRN2", target_bir_lowering=False, debug=False, num_devices=NCORES)

    # ---- DRAM I/O ----
    Qd = nc.dram_tensor("q_tok", [TQ, D], F32, kind="ExternalInput").ap()
    Kd = nc.dram_tensor("k_tok", [TKV, D], BF16, kind="ExternalInput").ap()
    Vd = nc.dram_tensor("v_tok", [TKV, D], BF16, kind="ExternalInput").ap()
    wq_d = nc.dram_tensor("wq", [D, INNER], BF16, kind="ExternalInput").ap()
    wk_d = nc.dram_tensor("wk", [D, INNER], BF16, kind="ExternalInput").ap()
    wv_d = nc.dram_tensor("wv", [D, INNER], BF16, kind="ExternalInput").ap()
    wo_d = nc.dram_tensor("wo", [INNER, D], BF16, kind="ExternalInput").ap()
    w1_d = nc.dram_tensor("w1", [P, MC * DC * P], FP8, kind="ExternalInput").ap()
    w2_d = nc.dram_tensor("w2", [P, (MC // 2) * 2 * D], FP8, kind="ExternalInput").ap()
    bff1_d = None
    if np.any(bias_rows["bff1"]):
        bff1_d = nc.dram_tensor("bff1", [P, MC], F32, kind="ExternalInput").ap()
    brow_d = {}
    for name in ("bq", "bk", "bv", "bo", "b2"):
        if np.any(bias_rows[name]):
            brow_d[name] = nc.dram_tensor(
                "brow_" + name, [1, bias_rows[name].shape[0]], BF16,
                kind="ExternalInput",
            ).ap()
    m_dram = nc.dram_tensor("m_ar", [P, IC * DH], F32).ap()
    Yd = nc.dram_tensor("y", [TQ, D], F32, kind="ExternalOutput").ap()

    Qt = Qd.rearrange("(t p) d -> t p d", p=P)
    Kt = Kd.rearrange("(t p) d -> t p d", p=P)
    Vt = Vd.rearrange("(t p) d -> t p d", p=P)
    Yt = Yd.rearrange("(t p) d -> t p d", p=P)
    wq_v = wq_d.rearrange("(c p) n -> p c n", p=P)
    wk_v = wk_d.rearrange("(c p) n -> p c n", p=P)
    wv_v = wv_d.rearrange("(c p) n -> p c n", p=P)
    wo_v = wo_d.rearrange("(c p) n -> p c n", p=P)

    with tile.TileContext(nc) as tc:
        with tc.tile_pool(name="singles", bufs=1) as singles:
            # ---- resident state ----
            wq_sb = singles.tile([P, DC, INNER], BF16)
            q_in = singles.tile([P, NQT, D], F32)
            xnT4 = singles.tile([P, DC, TQ], FP8)
            eps_tile = singles.tile([P, 1], F32)
            nc.vector.memset(eps_tile[:], EPS)
            # ffn ln runs on s2-scaled x with sx fold: rs = sx/sqrt(var+s2^2*eps)
            epsf_tile = singles.tile([P, 1], F32)
            nc.vector.memset(epsf_tile[:], EPS * s2 * s2 / (SX * SX))
            ones_row = singles.tile([1, P], BF16)
            nc.vector.memset(ones_row[:], 1.0)
            brow_sb = {}
            for name, ap in brow_d.items():
                t = singles.tile([1, ap.shape[1]], BF16, tag="brow_" + name)
                nc.sync.dma_start(t[:], ap[:])
                brow_sb[name] = t
            bff1_sb = None
            if bff1_d is not None:
                bff1_sb = singles.tile([P, MC], F32)
                nc.sync.dma_start(bff1_sb[:], bff1_d[:])
            M_sb = singles.tile([P, IC, P], BF16)
            nc.vector.memset(M_sb[:], 0.0)
            m_sb = singles.tile([P, IC, DH], F32)
            mr_sb = singles.tile([P, IC, DH], F32)


            def bias_mm(ps, name, lo, hi):
                if name in brow_sb:
                    nc.tensor.matmul(
                        ps,
                        ones_row[:, : ps.shape[0]],
                        brow_sb[name][:, lo:hi],
                        start=False,
                        stop=True,
                        skip_group_check=True,
                    )
                    return True
                return False

            # ---------------- Phase 1: KV half -> partial M ----------------
            with (
                tc.tile_pool(name="kv_w", bufs=1) as kv_w,
                tc.tile_pool(name="kv_io", bufs=3) as kv_io,
                tc.tile_pool(name="kv_mid", bufs=3) as kv_mid,
                tc.tile_pool(name="kv_stats", bufs=4) as kv_stats,
                tc.tile_pool(name="kv_ps", bufs=4, space="PSUM") as kv_ps,
                tc.tile_pool(name="m_ps", bufs=1, space="PSUM") as m_ps_pool,
            ):
                wk_sb = kv_w.tile([P, DC, INNER], BF16)
                wv_sb = kv_w.tile([P, DC, INNER], BF16)
                for c in range(2):
                    nc.sync.dma_start(wk_sb[:, c, :], wk_v[:, c, :])
                for c in range(2, DC):
                    nc.scalar.dma_start(wk_sb[:, c, :], wk_v[:, c, :])
                nc.gpsimd.dma_start(wv_sb[:], wv_v[:])
                # bulk loads for later phases (gpsimd SWDGE queues, after wv)
                for t in range(NQT):
                    nc.gpsimd.dma_start(q_in[:, t, :], Qt[t][:])
                nc.gpsimd.dma_start(wq_sb[:], wq_v[:])
                M_ps = m_ps_pool.tile([P, IC, P], F32)
                for t in range(NKT):
                    kn_bf = None
                    v_bf = None
                    for which in ("k", "v"):
                        src = Kt[t] if which == "k" else Vt[t]
                        w_sb = wk_sb if which == "k" else wv_sb
                        bname = "bk" if which == "k" else "bv"
                        x_in = kv_io.tile([P, D], BF16, tag="kv_in")
                        nc.sync.dma_start(x_in[:], src[:])
                        rs, nmu = _ln_stats_ops(nc, kv_stats, x_in, D, eps_tile, 1.0)
                        xn = kv_mid.tile([P, D], BF16, tag="kv_std")
                        nc.scalar.activation(
                            out=xn[:], in_=x_in[:], func=AF.Identity, bias=nmu[:], scale=rs[:]
                        )
                        xnT = kv_mid.tile([P, DC, P], BF16, tag="kv_xnT")
                        for c in range(DC):
                            nc.sync.dma_start(
                                xnT[:, c, :], xn[:, c * P : (c + 1) * P], transpose=True
                            )
                        pss = [kv_ps.tile([P, 512], F32, tag="kv_proj", name=f"kvp{t}_{which}_{g}") for g in range(2)]
                        for c in range(DC):
                            for g in range(2):
                                nc.tensor.matmul(
                                    pss[g][:],
                                    xnT[:, c, :],
                                    w_sb[:, c, g * 512 : (g + 1) * 512],
                                    start=(c == 0),
                                    stop=(c == DC - 1) and (bname not in brow_sb),
                                )
                        for g in range(2):
                            bias_mm(pss[g][:], bname, g * 512, (g + 1) * 512)
                        if which == "v":
                            v_bf = kv_mid.tile([P, INNER], BF16, tag="v_bf")
                            for g in range(2):
                                nc.scalar.activation(
                                    out=v_bf[:, g * 512 : (g + 1) * 512],
                                    in_=pss[g][:],
                                    func=AF.Copy,
                                )
                        else:
                            kn_bf = kv_mid.tile([P, H, DH], BF16, tag="kn_bf")
                            _l2_ops(nc, kv_mid, kv_stats, pss, kn_bf, mybir)
                    kn_flat = kn_bf.rearrange("p h f -> p (h f)")
                    for pr in range(IC):
                        nc.tensor.matmul(
                            M_ps[:, pr, :],
                            kn_flat[:, pr * P : (pr + 1) * P],
                            v_bf[:, pr * P : (pr + 1) * P],
                            start=(t == 0 and pr % 4 == 0),
                            stop=(t == NKT - 1 and pr % 4 == 3),
                            skip_group_check=True,
                        )
                # evac the per-head diag blocks: head pair pr holds
                # M_2pr at [0:64, pr, 0:64] and M_2pr+1 at [64:128, pr, 64:128]
                for po in (0, 64):
                    nc.scalar.activation(
                        out=m_sb[po : po + 64, :, :],
                        in_=M_ps[po : po + 64, :, po : po + 64],
                        func=AF.Copy,
                    )
            # pair all-reduce of the partial M
            nc.sync.dma_start(m_dram.rearrange("p (c f) -> p c f", c=IC)[:], m_sb[:])
            nc.gpsimd.collective_compute(
                "AllReduce",
                ALU.add,
                replica_groups=[[0, 1], [2, 3], [4, 5], [6, 7]],
                ins=[m_dram[:]],
                outs=[m_dram[:]],
            )
            nc.sync.dma_start(mr_sb[:], m_dram.rearrange("p (c f) -> p c f", c=IC)[:])
            for po in (0, 64):
                nc.vector.tensor_copy(
                    out=M_sb[po : po + 64, :, po : po + 64],
                    in_=mr_sb[po : po + 64, :, :],
                )

            # ---------------- Phase 2+3: Q -> attn -> x (+ FFN prep) --------
            with tc.tile_pool(name="wide", bufs=1) as wide:
                w2_sb = wide.tile([P, MC // 2, 2, D], FP8)
                w2f = w2_sb.rearrange("p a b c -> p (a b c)")
                CH = 4096
                for i in range((MC // 2) * 2 * D // CH):
                    nc.scalar.dma_start(
                        w2f[:, i * CH : (i + 1) * CH], w2_d[:, i * CH : (i + 1) * CH]
                    )

                with (
                    tc.tile_pool(name="q_w", bufs=1) as q_w,
                    tc.tile_pool(name="qn2", bufs=NQT) as qn2_pool,
                    tc.tile_pool(name="q_mid", bufs=3) as q_mid,
                    tc.tile_pool(name="q_stats", bufs=4) as q_stats,
                    tc.tile_pool(name="q_ps", bufs=4, space="PSUM") as q_ps,
                    tc.tile_pool(name="at_ps", bufs=1, space="PSUM") as at_ps,
                    tc.tile_pool(name="x_ps", bufs=2, space="PSUM") as x_ps,
                ):
                    wo_sb = q_w.tile([P, IC, D], BF16)
                    for c in range(0, IC, 2):
                        nc.sync.dma_start(wo_sb[:, c : c + 2, :], wo_v[:, c : c + 2, :])
                    # ---- stage 1: LN1 + wq proj + l2 norm (AR-independent) ----
                    qn2_list = []
                    for t in range(NQT):
                        rs, nmu = _ln_stats_ops(
                            nc, q_stats, q_in[:, t, :], D, eps_tile, 1.0
                        )
                        qx = q_mid.tile([P, D], BF16, tag="q_std")
                        nc.scalar.activation(
                            out=qx[:], in_=q_in[:, t, :], func=AF.Identity,
                            bias=nmu[:], scale=rs[:],
                        )
                        qxT = q_mid.tile([P, DC, P], BF16, tag="q_xnT")
                        for c in range(DC):
                            nc.sync.dma_start(
                                qxT[:, c, :], qx[:, c * P : (c + 1) * P], transpose=True
                            )
                        pss = [q_ps.tile([P, 512], F32, tag="q_proj", name=f"qp{t}_{g}") for g in range(2)]
                        for c in range(DC):
                            for g in range(2):
                                nc.tensor.matmul(
                                    pss[g][:],
                                    qxT[:, c, :],
                                    wq_sb[:, c, g * 512 : (g + 1) * 512],
                                    start=(c == 0),
                                    stop=(c == DC - 1) and ("bq" not in brow_sb),
                                )
                        for g in range(2):
                            bias_mm(pss[g][:], "bq", g * 512, (g + 1) * 512)
                        qn_bf = q_mid.tile([P, H, DH], BF16, tag="qn_bf")
                        _l2_ops(nc, q_mid, q_stats, pss, qn_bf, mybir)
                        qn_flat = qn_bf.rearrange("p h f -> p (h f)")
                        qnT2 = qn2_pool.tile([P, IC, P], BF16, tag="qnT2", name=f"qnT2_{t}")
                        for c in range(IC):
                            nc.sync.dma_start(
                                qnT2[:, c, :], qn_flat[:, c * P : (c + 1) * P],
                                transpose=True,
                            )
                        qn2_list.append(qnT2)
                    # ---- stage 2+3: attn apply + wo + residual + FFN prep ----
                    for t in range(NQT):
                        qnT2 = qn2_list[t]
                        a_ps = at_ps.tile([P, IC, P], F32, tag="attn_ps")
                        for pr in range(IC):
                            nc.tensor.matmul(
                                a_ps[:, pr, :],
                                M_sb[:, pr, :],
                                qnT2[:, pr, :],
                                start=True,
                                stop=True,
                                skip_group_check=True,
                            )
                        aT_bf = q_mid.tile([P, IC, P], BF16, tag="aT_bf")
                        nc.scalar.activation(out=aT_bf[:], in_=a_ps[:], func=AF.Copy)
                        xps = [x_ps.tile([P, 512], F32, tag="x_proj", name=f"xp{t}_{g}") for g in range(2)]
                        for c in range(IC):
                            for g in range(2):
                                nc.tensor.matmul(
                                    xps[g][:],
                                    aT_bf[:, c, :],
                                    wo_sb[:, c, g * 512 : (g + 1) * 512],
                                    start=(c == 0),
                                    stop=(c == IC - 1) and ("bo" not in brow_sb),
                                )
                        for g in range(2):
                            bias_mm(xps[g][:], "bo", g * 512, (g + 1) * 512)
                            nc.vector.tensor_tensor(
                                out=q_in[:, t, g * 512 : (g + 1) * 512],
                                in0=xps[g][:],
                                in1=q_in[:, t, g * 512 : (g + 1) * 512],
                                op=ALU.add,
                            )
                        # FFN prep for this tile: ln2 + transpose + fp8 cast
                        rs2, nmu2 = _ln_stats_ops(
                            nc, q_stats, q_in[:, t, :], D, epsf_tile, 1.0 / (SX * SX)
                        )
                        fx = q_mid.tile([P, D], BF16, tag="f_std")
                        nc.scalar.activation(
                            out=fx[:], in_=q_in[:, t, :], func=AF.Identity,
                            bias=nmu2[:], scale=rs2[:],
                        )
                        fxT = q_mid.tile([P, DC, P], BF16, tag="f_xnT")
                        for c in range(DC):
                            nc.sync.dma_start(
                                fxT[:, c, :], fx[:, c * P : (c + 1) * P], transpose=True
                            )
                        nc.vector.tensor_copy(
                            out=xnT4[:, :, t * P : (t + 1) * P], in_=fxT[:]
                        )

                # ---------------- Phase 4: FFN (fp8 DoubleRow) ----------------
                with (
                    tc.tile_pool(name="f_h", bufs=1) as f_h,
                    tc.tile_pool(name="f_w", bufs=4) as f_w,
                    tc.tile_pool(name="f_out", bufs=3) as f_out,
                ):
                    h4 = f_h.tile([P, MC, TQ], FP8)
                    MSTR = 2 * (DC // 2) * 2 * P  # dram columns per m-pair
                    with tc.tile_pool(name="h_ps", bufs=2, space="PSUM") as h_ps:
                      for mp in range(MC // 2):
                        w1t = f_w.tile([P, 2, DC // 2, 2, P], FP8, tag="w1t")
                        nc.gpsimd.dma_start(
                            w1t.rearrange("p a b c d -> p (a b c d)")[:],
                            w1_d[:, mp * MSTR : (mp + 1) * MSTR],
                        )
                        hp2 = h_ps.tile([P, 2, TQ], F32, tag="h_ps_t")
                        for jm in range(2):
                            m = 2 * mp + jm
                            for hh in range(2):
                                hsl = slice(hh * 512, (hh + 1) * 512)
                                for c2 in range(DC // 2):
                                    nc.tensor.matmul(
                                        hp2[:, jm, hsl],
                                        w1t[:, jm, c2, :, :],
                                        xnT4[:, 2 * c2 : 2 * c2 + 2, hsl],
                                        start=(c2 == 0),
                                        stop=(c2 == DC // 2 - 1),
                                        perf_mode=mybir.MatmulPerfMode.DoubleRow,
                                    )
                        if bff1_sb is not None:
                            for jm in range(2):
                                m = 2 * mp + jm
                                nc.scalar.activation(
                                    out=h4[:, m, :],
                                    in_=hp2[:, jm, :],
                                    func=AF.Gelu,
                                    bias=bff1_sb[:, m : m + 1],
                                    scale=gelu_scale,
                                )
                        else:
                            nc.scalar.activation(
                                out=h4[:, 2 * mp : 2 * mp + 2, :],
                                in_=hp2[:],
                                func=AF.Gelu,
                                bias=0.0,
                                scale=gelu_scale,
                            )
                    with tc.tile_pool(name="y_ps", bufs=4, space="PSUM") as y_ps:
                      for t in range(NQT):
                        for g in range(2):
                            yp = y_ps.tile([P, 512], F32, tag="y_ps_t")
                            for m2 in range(MC // 2):
                                nc.tensor.matmul(
                                    yp[:],
                                    h4[:, 2 * m2 : 2 * m2 + 2, t * P : (t + 1) * P],
                                    w2_sb[:, m2, :, g * 512 : (g + 1) * 512],
                                    start=(m2 == 0),
                                    stop=(m2 == MC // 2 - 1) and ("b2" not in brow_sb),
                                    perf_mode=mybir.MatmulPerfMode.DoubleRow,
                                )
                            bias_mm(yp[:], "b2", g * 512, (g + 1) * 512)
                            y_out = f_out.tile([P, 512], F32, tag="y_out")
                            nc.vector.tensor_tensor(
                                out=y_out[:],
                                in0=yp[:],
                                in1=q_in[:, t, g * 512 : (g + 1) * 512],
                                op=ALU.add,
                            )
                            nc.scalar.dma_start(
                                Yt[t][:, g * 512 : (g + 1) * 512], y_out[:]
                            )

    nc.compile()
    return nc


def _pow2_scale(arr, target=224.0):
    m = float(np.abs(arr).max())
    if m == 0:
        return 1.0
    return float(2.0 ** np.floor(np.log2(target / m)))


def prep_inputs(inputs):
    """Host-side shard + weight folding. Returns (in_maps, bias_rows, s1, s2)."""
    f32 = np.float32
    bf = ml_dtypes.bfloat16
    f8 = ml_dtypes.float8_e4m3
    g1 = np.asarray(inputs["ln1_g"], f32)
    b1ln = np.asarray(inputs["ln1_b"], f32)
    g2 = np.asarray(inputs["ln2_g"], f32)
    b2ln = np.asarray(inputs["ln2_b"], f32)
    wq = np.asarray(inputs["wq"], f32)
    wk = np.asarray(inputs["wk"], f32)
    wv = np.asarray(inputs["wv"], f32)
    wo = np.asarray(inputs["wo"], f32)
    w1 = np.asarray(inputs["w1"], f32)
    w2 = np.asarray(inputs["w2"], f32)

    w1g = g2[:, None] * w1
    s1 = _pow2_scale(w1g)
    s2 = _pow2_scale(w2)

    bias_rows = {
        "bq": (b1ln @ wq).astype(f32),
        "bk": (b1ln @ wk).astype(f32),
        "bv": (b1ln @ wv).astype(f32),
        "bo": (np.asarray(inputs["bo"], f32) * s2),
        "b2": (np.asarray(inputs["b2"], f32) * s2),
        "bff1": (b2ln @ w1 + np.asarray(inputs["b1"], f32)).astype(f32),
    }
    bff1_tile = np.ascontiguousarray(bias_rows["bff1"].reshape(MC, P).T)

    wq_b = np.ascontiguousarray((g1[:, None] * wq).astype(bf))
    wk_b = np.ascontiguousarray((g1[:, None] * wk).astype(bf))
    wv_b = np.ascontiguousarray((g1[:, None] * wv).astype(bf))
    wo_b = np.ascontiguousarray((wo * s2).astype(bf))
    # w1 packed for DoubleRow: [p, m, c2, i, q] = (s1*w1g)[(2c2+i)*128+p, m*128+q]
    w1s = (w1g * s1).astype(f8).astype(f32)
    w1p = (
        w1s.reshape(DC // 2, 2, P, MC, P)  # [c2, i, p, m, q]
        .transpose(2, 3, 0, 1, 4)  # [p, m, c2, i, q]
        .reshape(P, MC * DC * P)
    )
    w1_b = np.ascontiguousarray(w1p.astype(f8))
    # w2 packed for DoubleRow: [p, m2, i, of] = (s2*w2)[(2m2+i)*128+p, of]
    w2s = (w2 * s2).astype(f8).astype(f32)
    w2p = (
        w2s.reshape(MC // 2, 2, P, D)  # [m2, i, p, of]
        .transpose(2, 0, 1, 3)  # [p, m2, i, of]
        .reshape(P, (MC // 2) * 2 * D)
    )
    w2_b = np.ascontiguousarray(w2p.astype(f8))

    Q = np.asarray(inputs["Q"], f32) * s2
    K = np.asarray(inputs["K"], f32)
    V = np.asarray(inputs["V"], f32)

    in_maps = []
    for c in range(NCORES):
        b = c // 2
        r0 = (c % 2) * TQ
        m = {
            "q_tok": np.ascontiguousarray(Q[b, r0 : r0 + TQ]),
            "k_tok": np.ascontiguousarray(K[b, r0 : r0 + TKV].astype(bf)),
            "v_tok": np.ascontiguousarray(V[b, r0 : r0 + TKV].astype(bf)),
            "wq": wq_b,
            "wk": wk_b,
            "wv": wv_b,
            "wo": wo_b,
            "w1": w1_b,
            "w2": w2_b,
        }
        if np.any(bias_rows["bff1"]):
            m["bff1"] = bff1_tile
        for name in ("bq", "bk", "bv", "bo", "b2"):
            if np.any(bias_rows[name]):
                m["brow_" + name] = bias_rows[name][None, :].astype(bf)
        in_maps.append(m)
    return in_maps, bias_rows, s1, s2


def gather(results, s2):
    out = np.empty((B, N, D), np.float32)
    inv = 1.0 / s2
    for c in range(NCORES):
        b = c // 2
        r0 = (c % 2) * TQ
        out[b, r0 : r0 + TQ] = results[c]["y"] * inv
    return out


_NC_CACHE = {}


def get_nc(bias_rows, s1, s2):
    key = (tuple(sorted(n for n, r in bias_rows.items() if np.any(r))), s1, s2)
    if key not in _NC_CACHE:
        _NC_CACHE[key] = build_nc(bias_rows, s2, 1.0 / (s1 * SX))
    return _NC_CACHE[key]


def kernel(**inputs) -> np.ndarray:
    from concourse.bass_utils import run_bass_kernel_spmd

    in_maps, bias_rows, s1, s2 = prep_inputs(inputs)
    nc = get_nc(bias_rows, s1, s2)
    res = run_bass_kernel_spmd(nc, in_maps, core_ids=list(range(NCORES)))
    return gather(res.results, s2)
